# revision 18
# baseline (speedup 1.0000x reference)
"""Trainium2 Bass kernel for multi-head attention (B=8,S=1024,D=768,H=12).

Sharding: pure data-parallel over batch (B=8 == n_cores=8). Each core runs
the full attention for one batch element; no collectives needed.

Per-core pipeline (all matmuls fp32r, moving-dim 512):
  1. Host passes q,k,v pre-transposed [D,S]; weights augmented with bias row.
  2. Projections: qhT/khT = [D,S] layout; vh = v@Wv in [S,D] with a ones
     column appended per head (65-wide groups) so AV yields row-sums free.
  3. Natural scores (for the attn output): psum = I@bias + qhT^T@khT,
     ACT exp with fused row-sum accum, DVE reciprocal + scale -> attn.
  4. Transposed scores (for A@V): psumT = I@biasT + khT^T@qhT, ACT exp
     -> expLT in SBUF; AV: O^T[65,S] += vh_aug^T @ expLT (row 64 = rowsum).
     Normalize O^T by DMA-broadcast reciprocal row, DVE multiply -> concatT.
  5. Output projection in natural layout from concatT, bias via ones-row.
"""

import sys

for _p in ("/opt/trn_rl_repo",):
    if _p not in sys.path:
        sys.path.insert(0, _p)

import numpy as np
from contextlib import ExitStack

import concourse.bass as bass
import concourse.tile as tile
from concourse import bacc, mybir
from concourse.bass_utils import run_bass_kernel_spmd

FP = mybir.dt.float32
FPR = mybir.dt.float32r
AF = mybir.ActivationFunctionType

B, S, D, H = 8, 1024, 768, 12
DEPTH = D // H  # 64
HW = DEPTH + 1  # 65: head width in vh_aug (64 + ones column)
P = 128
NS = S // P  # 8 sequence tiles
ND = D // P  # 6 feature tiles

_NC_CACHE = {}


def _build_nc(with_proj_bias):
    nc = bacc.Bacc()

    qT_d = nc.declare_dram_parameter("qT", [D, S], FP, isOutput=False)
    kT_d = nc.declare_dram_parameter("kT", [D, S], FP, isOutput=False)
    vT_d = nc.declare_dram_parameter("vT", [D, S], FP, isOutput=False)
    bias_d = nc.declare_dram_parameter("bias", [S, S], FP, isOutput=False)
    biasT_d = nc.declare_dram_parameter("biasT", [S, S], FP, isOutput=False)
    ident_d = nc.declare_dram_parameter("ident", [P, P], FP, isOutput=False)
    ones_d = nc.declare_dram_parameter("ones", [P, 16], FP, isOutput=False)
    onesrow_d = nc.declare_dram_parameter("onesrow", [1, S], FP, isOutput=False)
    wq_d = nc.declare_dram_parameter("wq", [D + 1, D], FP, isOutput=False)
    wk_d = nc.declare_dram_parameter("wk", [D + 1, D], FP, isOutput=False)
    wv_d = nc.declare_dram_parameter("wv", [D + 1, D], FP, isOutput=False)
    wo_d = nc.declare_dram_parameter("wo", [D + 1, D], FP, isOutput=False)
    out_d = nc.declare_dram_parameter("out", [S, D], FP, isOutput=True)
    attn_d = nc.declare_dram_parameter("attn", [H, S, S], FP, isOutput=True)
    rscratch_d = nc.dram_tensor("rscratch", [H, S], FP)

    with tile.TileContext(nc) as tc, ExitStack() as ctx:
        const = ctx.enter_context(tc.tile_pool(name="const", bufs=1))
        persist = ctx.enter_context(tc.tile_pool(name="persist", bufs=1))

        id_r = const.tile([P, P], FPR)
        nc.sync.dma_start(out=id_r, in_=ident_d[:, :].bitcast(FPR))
        ones_r = const.tile([P, 16], FPR)
        nc.sync.dma_start(out=ones_r, in_=ones_d[:, :].bitcast(FPR))
        onesrow_r = const.tile([1, S], FPR)
        nc.sync.dma_start(out=onesrow_r, in_=onesrow_d[:, :].bitcast(FPR))

        qhT = persist.tile([P, ND, S], FPR)  # 24KB/part
        khT = persist.tile([P, ND, S], FPR)  # 24KB/part
        vh_aug = persist.tile([P, NS, H * HW], FPR)  # 24.4KB/part

        # ---------------- Phase A: projections ----------------
        with (
            tc.tile_pool(name="wpool", bufs=2) as wpool,
            tc.tile_pool(name="xT", bufs=2) as xT_pool,
            tc.tile_pool(name="psp", bufs=2, space="PSUM") as psp_pool,
        ):
            for ti, (x_d, w_d, dst) in enumerate(
                ((qT_d, wq_d, qhT), (kT_d, wk_d, khT), (vT_d, wv_d, vh_aug))
            ):
                w_sb = wpool.tile([P, ND, D], FPR, tag="w")
                for kc in range(ND):
                    nc.sync.dma_start(
                        out=w_sb[:, kc, :],
                        in_=w_d[kc * P:(kc + 1) * P, :].bitcast(FPR),
                    )
                if with_proj_bias:
                    wb_sb = wpool.tile([1, D], FPR, tag="wb")
                    nc.sync.dma_start(out=wb_sb, in_=w_d[D:D + 1, :].bitcast(FPR))

                xT = xT_pool.tile([P, ND, S], FPR, tag="xT")
                for kc in range(ND):
                    nc.sync.dma_start(
                        out=xT[:, kc, :],
                        in_=x_d[kc * P:(kc + 1) * P, :].bitcast(FPR),
                    )

                if dst is vh_aug:
                    # vh natural [S, D]: psum[s,dout] = sum_kc vT_kc_sblock^T @ wv_kc
                    for st in range(NS):
                        pp = psp_pool.tile([P, D], FP, tag="pp")
                        for n0, n1 in ((0, 512), (512, 768)):
                            for kc in range(ND):
                                nc.tensor.matmul(
                                    pp[:, n0:n1],
                                    xT[:, kc, st * P:(st + 1) * P],
                                    w_sb[:, kc, n0:n1],
                                    start=(kc == 0),
                                    stop=(kc == ND - 1 and not with_proj_bias),
                                )
                            if with_proj_bias:
                                nc.tensor.matmul(
                                    pp[:, n0:n1],
                                    onesrow_r[0:1, 0:P],
                                    wb_sb[0:1, n0:n1],
                                    start=False,
                                    stop=True,
                                )
                        # strided copy into 65-wide head groups
                        dview = dst[:, st, :].rearrange("p (h e) -> p h e", h=H)
                        eng_v = st % 2 == 0
                        if eng_v:
                            nc.vector.tensor_copy(
                                out=dview[:, :, 0:DEPTH],
                                in_=pp.rearrange("p (h e) -> p h e", e=DEPTH),
                            )
                        else:
                            nc.scalar.activation(
                                out=dview[:, :, 0:DEPTH],
                                in_=pp.rearrange("p (h e) -> p h e", e=DEPTH),
                                func=AF.Copy,
                            )
                        nc.vector.tensor_copy(
                            out=dview[:, :, DEPTH:HW],
                            in_=ones_r[:, 0:H].rearrange("p (h o) -> p h o", o=1),
                        )
                else:
                    for dt_ in range(ND):
                        pp = psp_pool.tile([P, S], FP, tag="pp")
                        for half in range(2):
                            n0, n1 = half * 512, (half + 1) * 512
                            for kc in range(ND):
                                nc.tensor.matmul(
                                    pp[:, n0:n1],
                                    w_sb[:, kc, dt_ * P:(dt_ + 1) * P],
                                    xT[:, kc, n0:n1],
                                    start=(kc == 0),
                                    stop=(kc == ND - 1 and not with_proj_bias),
                                )
                            if with_proj_bias:
                                nc.tensor.matmul(
                                    pp[:, n0:n1],
                                    wb_sb[0:1, dt_ * P:(dt_ + 1) * P],
                                    onesrow_r[0:1, n0:n1],
                                    start=False,
                                    stop=True,
                                )
                        if dt_ % 2 == 0:
                            nc.vector.tensor_copy(out=dst[:, dt_, :], in_=pp)
                        else:
                            nc.scalar.activation(out=dst[:, dt_, :], in_=pp, func=AF.Copy)

        # bias/biasT and concatT live from here to the end; allocated after
        # phase-A pools close so they reuse that SBUF space.
        persist2 = ctx.enter_context(tc.tile_pool(name="persist2", bufs=1))
        bias_sb = persist2.tile([P, NS, S], FPR)  # 32KB/part
        biasT_sb = persist2.tile([P, NS, S], FPR)  # 32KB/part
        concatT = persist2.tile([P, ND, S], FPR)  # 24KB/part
        for st in range(NS):
            nc.sync.dma_start(
                out=bias_sb[:, st, :],
                in_=bias_d[st * P:(st + 1) * P, :].bitcast(FPR),
            )
            nc.sync.dma_start(
                out=biasT_sb[:, st, :],
                in_=biasT_d[st * P:(st + 1) * P, :].bitcast(FPR),
            )

        # ---------------- Phase B: attention ----------------
        with (
            tc.tile_pool(name="psl", bufs=2, space="PSUM") as psl_pool,
            tc.tile_pool(name="pslt", bufs=1, space="PSUM") as pslt_pool,
            tc.tile_pool(name="pso", bufs=1, space="PSUM") as pso_pool,
            tc.tile_pool(name="attn", bufs=3) as attn_pool,
            tc.tile_pool(name="explt", bufs=2) as explt_pool,
            tc.tile_pool(name="rb", bufs=1) as rb_pool,
            tc.tile_pool(name="small", bufs=4) as small_pool,
        ):
            for h in range(H):
                t_h = h // 2
                r0 = (h % 2) * DEPTH

                # natural side: attn output
                for st in range(NS):
                    psl = psl_pool.tile([P, S], FP, tag="psl")
                    for half in range(2):
                        n0, n1 = half * 512, (half + 1) * 512
                        nc.tensor.matmul(
                            psl[:, n0:n1],
                            id_r,
                            bias_sb[:, st, n0:n1],
                            start=True,
                            stop=False,
                        )
                        nc.tensor.matmul(
                            psl[:, n0:n1],
                            qhT[r0:r0 + DEPTH, t_h, st * P:(st + 1) * P],
                            khT[r0:r0 + DEPTH, t_h, n0:n1],
                            start=False,
                            stop=True,
                        )
                    at_t = attn_pool.tile([P, S], FP, tag="attn")
                    rowsum = small_pool.tile([P, 1], FP, tag="rs")
                    nc.scalar.activation(
                        out=at_t, in_=psl, func=AF.Exp, accum_out=rowsum
                    )
                    recip = small_pool.tile([P, 1], FP, tag="rc")
                    nc.vector.reciprocal(out=recip, in_=rowsum)
                    nc.vector.tensor_scalar_mul(at_t, at_t, recip)
                    nc.sync.dma_start(
                        out=attn_d[h, st * P:(st + 1) * P, :], in_=at_t
                    )

                # transposed side: A^T and AV
                pso = pso_pool.tile([HW, S], FP, tag="pso")
                for i in range(NS):
                    pslt = pslt_pool.tile([P, S], FP, tag="pslt")
                    for half in range(2):
                        n0, n1 = half * 512, (half + 1) * 512
                        nc.tensor.matmul(
                            pslt[:, n0:n1],
                            id_r,
                            biasT_sb[:, i, n0:n1],
                            start=True,
                            stop=False,
                        )
                        nc.tensor.matmul(
                            pslt[:, n0:n1],
                            khT[r0:r0 + DEPTH, t_h, i * P:(i + 1) * P],
                            qhT[r0:r0 + DEPTH, t_h, n0:n1],
                            start=False,
                            stop=True,
                        )
                    explt = explt_pool.tile([P, S], FPR, tag="explt")
                    nc.scalar.activation(out=explt, in_=pslt, func=AF.Exp)
                    for half in range(2):
                        n0, n1 = half * 512, (half + 1) * 512
                        nc.tensor.matmul(
                            pso[:, n0:n1],
                            vh_aug[:, i, h * HW:(h + 1) * HW],
                            explt[:, n0:n1],
                            start=(i == 0),
                            stop=(i == NS - 1),
                        )

                # normalize O^T rows by broadcast reciprocal of rowsum row
                rrow = small_pool.tile([1, S], FP, tag="rrow")
                nc.vector.reciprocal(out=rrow, in_=pso[DEPTH:HW, :])
                nc.gpsimd.dma_start(out=rscratch_d[h:h + 1, :], in_=rrow)
                rb = rb_pool.tile([DEPTH, S], FP, tag="rb")
                _sl = rscratch_d[h:h + 1, :]
                rrow_b = bass.AP(
                    tensor=_sl.tensor, offset=_sl.offset, ap=[[0, DEPTH], [1, S]]
                )
                nc.gpsimd.dma_start(out=rb, in_=rrow_b)
                nc.vector.tensor_mul(
                    out=concatT[r0:r0 + DEPTH, t_h, :],
                    in0=pso[0:DEPTH, :],
                    in1=rb,
                )

        # ---------------- Phase C: output projection ----------------
        if True:
            with (
                tc.tile_pool(name="wo", bufs=1) as wo_pool,
                tc.tile_pool(name="osb", bufs=3) as out_pool,
                tc.tile_pool(name="psq", bufs=2, space="PSUM") as psq_pool,
            ):
                wo_sb = wo_pool.tile([P, ND, D], FPR, tag="wo")
                for kc in range(ND):
                    nc.sync.dma_start(
                        out=wo_sb[:, kc, :],
                        in_=wo_d[kc * P:(kc + 1) * P, :].bitcast(FPR),
                    )
                if with_proj_bias:
                    wob_sb = wo_pool.tile([1, D], FPR, tag="wob")
                    nc.sync.dma_start(out=wob_sb, in_=wo_d[D:D + 1, :].bitcast(FPR))

                for st in range(NS):
                    pq = psq_pool.tile([P, D], FP, tag="pq")
                    for n0, n1 in ((0, 512), (512, 768)):
                        for kc in range(ND):
                            nc.tensor.matmul(
                                pq[:, n0:n1],
                                concatT[:, kc, st * P:(st + 1) * P],
                                wo_sb[:, kc, n0:n1],
                                start=(kc == 0),
                                stop=(kc == ND - 1 and not with_proj_bias),
                            )
                        if with_proj_bias:
                            nc.tensor.matmul(
                                pq[:, n0:n1],
                                onesrow_r[0:1, 0:P],
                                wob_sb[0:1, n0:n1],
                                start=False,
                                stop=True,
                            )
                    o_sb = out_pool.tile([P, D], FP, tag="osb")
                    if st % 2 == 0:
                        nc.vector.tensor_copy(out=o_sb, in_=pq)
                    else:
                        nc.scalar.activation(out=o_sb, in_=pq, func=AF.Copy)
                    nc.sync.dma_start(out=out_d[st * P:(st + 1) * P, :], in_=o_sb)

    nc.finalize()
    return nc


def _prep_in_maps(v, k, q, mask, adjoin_matrix, wq, bq, wk, bk, wv, bv, wo, bo):
    v = np.asarray(v, dtype=np.float32)
    k = np.asarray(k, dtype=np.float32)
    q = np.asarray(q, dtype=np.float32)
    mask = np.asarray(mask, dtype=np.float32)
    adjoin_matrix = np.asarray(adjoin_matrix, dtype=np.float32)
    wq = np.asarray(wq, dtype=np.float32)
    bq = np.asarray(bq, dtype=np.float32)
    wk = np.asarray(wk, dtype=np.float32)
    bk = np.asarray(bk, dtype=np.float32)
    wv = np.asarray(wv, dtype=np.float32)
    bv = np.asarray(bv, dtype=np.float32)
    wo = np.asarray(wo, dtype=np.float32)
    bo = np.asarray(bo, dtype=np.float32)

    scale = 1.0 / np.sqrt(np.float32(DEPTH))
    wq_aug = np.concatenate([wq * scale, (bq * scale)[None, :]], axis=0)
    wk_aug = np.concatenate([wk, bk[None, :]], axis=0)
    wv_aug = np.concatenate([wv, bv[None, :]], axis=0)
    wo_aug = np.concatenate([wo, bo[None, :]], axis=0)
    # bias[sq, sk] = adjoin[b,0,sq,sk] + (-1e9)*mask[b,0,0,sk]
    bias = adjoin_matrix[:, 0, :, :] + (-1e9) * mask[:, 0, 0, :][:, None, :]
    biasT = np.ascontiguousarray(bias.transpose(0, 2, 1))
    ident = np.eye(P, dtype=np.float32)
    ones = np.ones((P, 16), dtype=np.float32)
    onesrow = np.ones((1, S), dtype=np.float32)
    qT = np.ascontiguousarray(q.transpose(0, 2, 1))
    kT = np.ascontiguousarray(k.transpose(0, 2, 1))
    vT = np.ascontiguousarray(v.transpose(0, 2, 1))

    with_proj_bias = any(
        np.any(x != 0) for x in (bq, bk, bv, bo)
    )
    in_maps = [
        {
            "qT": qT[b],
            "kT": kT[b],
            "vT": vT[b],
            "bias": np.ascontiguousarray(bias[b]),
            "biasT": biasT[b],
            "ident": ident,
            "ones": ones,
            "onesrow": onesrow,
            "wq": wq_aug,
            "wk": wk_aug,
            "wv": wv_aug,
            "wo": wo_aug,
        }
        for b in range(B)
    ]
    return in_maps, with_proj_bias


def kernel(**inputs):
    in_maps, with_proj_bias = _prep_in_maps(**inputs)
    key = ("nc", with_proj_bias)
    if key not in _NC_CACHE:
        _NC_CACHE[key] = _build_nc(with_proj_bias)
    nc = _NC_CACHE[key]
    res = run_bass_kernel_spmd(nc, in_maps, list(range(B))).results
    output = np.stack([res[b]["out"] for b in range(B)], axis=0)
    attn = np.stack([res[b]["attn"] for b in range(B)], axis=0)
    return output, attn
